# revision 20
# baseline (speedup 1.0000x reference)
"""Trainium2 Bass kernel for nn_BetaVAEMark10Decoder.

Network (per sample): latent(4) -> Linear(256)+leaky -> reshape (1,8,32)
 -> convT(5,2)s(5,2) -> conv3x3 SAME +leaky   (5,16,16)
 -> convT(5,2)s(5,2) -> conv3x3 SAME +leaky   (25,32,8)
 -> convT(2,2)s(2,2) -> conv3x3 SAME +relu    (50,64,6)  -> NCHW out.

Each convT(k=s) + 3x3 conv pair composes into one exact linear map that is
block-banded over rows: output row y reads input rows i+d through per-phase
matrices R[p, d].  Everything becomes dense matmuls on 128-chunks.

v2: the final layer (L4, 2/3 of all PE cycles) runs as fp8e4m3 DoubleRow
matmuls at 0.5 cycles/row with the two row-contributions packed into the two
DR k-tiles, so one DR replaces two bf16 matmuls (4x MACs/cycle).  Precision
is restored by a 3-term compensated product
    x3 @ W  ~=  hi@Whi + lo@Whi + hi@Wlo
where (Whi, Wlo) is an exact host-side e4m3 split of the (scaled) weights
and (hi, lo) is an on-device split of x3 produced in 2 1/3 engine passes:
  ACT:  hi = e4m3(leaky(ps) * 15/16)     (psum is pre-scaled into fp8 domain
                                          by folding s3 into the bf16 w3)
  DVE:  lo_raw = ps - hi  -> e4m3        (on ps<0 lanes this is ~0.99*ps < 0)
  Pool: lo = max(lo_raw, 0)              (gates off the negative-lane junk;
                                          the 15/16 "floor shift" makes the
                                          true positive-lane residual >= 0)
The leaky'd negative lanes are ~1% magnitude, so hi alone carries them.
Measured end-to-end error 0.63% vs the fp32 reference (bf16 baseline 0.58%).

L4 row-pair edges use two zeroed pad rows in x3hi/x3lo instead of special
edge weights, so every output row is exactly one (pair, 3-group, 3-term)
DR set.  L3 rows and L4 row-pairs are emitted interleaved so the 19.7MB
output DMA (54.6us at 360GB/s, the largest single resource) spreads over
the whole kernel instead of piling into an L4-only tail.

Other levers kept from v1: bias folded into the L1 contraction as a 5th
ones-row, PE p-state warm-up matmuls under the first weight DMA, 8-bank
psum pool of [128,2,512] tiles, relu split across ACT and DVE (GPSIMD
cannot read PSUM) with the split ratio rebalanced for the DVE's new L3-sub
load, bf16 output (upcast + pow2 descale on host) flushed staggered by
batch-block parity.

Sharding: pure data parallel, batch 4096 -> 8 cores x 512.
"""

import sys

import numpy as np

sys.path.insert(0, "/opt/trn_rl_repo")

import ml_dtypes  # noqa: E402

import concourse.bass as bass  # noqa: E402
import concourse.bacc as bacc  # noqa: E402
import concourse.mybir as mybir  # noqa: E402
from concourse import tile  # noqa: E402
from concourse.bass_utils import run_bass_kernel_spmd  # noqa: E402

N_CORES = 8
B = 4096
BL = B // N_CORES  # 512 per core
F32 = mybir.dt.float32
F32R = mybir.dt.float32r
BF16 = mybir.dt.bfloat16
F8 = mybir.dt.float8e4
NPBF = ml_dtypes.bfloat16
NPF8 = (ml_dtypes.float8_e4m3fn if hasattr(ml_dtypes, "float8_e4m3fn")
        else ml_dtypes.float8_e4m3)
DR = mybir.MatmulPerfMode.DoubleRow

# L4 feature windows per column group: group g of an output row (cols
# 128g..128(g+1), x-major) only reads input feats within these windows.
L4_WIN = ((0, 128), (80, 176), (128, 256))


# ---------------------------------------------------------------- host math
def _fused_matrices(Wup, Wc, sy, sx, Win, in_idx, out_idx, n_out_cols):
    """Compose convT(k=s=(sy,sx)) with 3x3 SAME conv into per-phase row
    matrices.  Returns {(p, delta): M} where out row y (p = y%sy, i = y//sy)
    accumulates  in_row[i+delta] @ M[(p, delta)]  over available deltas.
    x-edge clipping is baked into M; y-edge clipping == skipping absent rows.
    """
    Wup = np.asarray(Wup, np.float32)
    Wc = np.asarray(Wc, np.float32)
    Cin = Wup.shape[2]
    Wout = Win * sx
    mats = {}
    for p in range(sy):
        deltas = {0}
        if p == 0:
            deltas.add(-1)
        if p == sy - 1:
            deltas.add(1)
        for d in sorted(deltas):
            M = np.zeros((Win * Cin, n_out_cols), np.float32)
            y = sy + p  # representative interior row
            i_t = y // sy + d
            nz = False
            for dy in (-1, 0, 1):
                yp = y + dy
                if yp // sy != i_t:
                    continue
                py = yp % sy
                for x in range(Wout):
                    for dx in (-1, 0, 1):
                        xp = x + dx
                        if xp < 0 or xp >= Wout:
                            continue
                        j, qx = divmod(xp, sx)
                        # conv_transpose (transpose_kernel=False) applies the
                        # spatially mirrored kernel per phase
                        CC = Wup[sy - 1 - py, sx - 1 - qx] @ Wc[dy + 1, dx + 1]
                        M[np.ix_(in_idx(j), out_idx(x))] += CC
                        nz = True
            if nz:
                mats[(p, d)] = M
    return mats


def build_host_matrices(W_lin, W_up1, W_c1, W_up2, W_c2, W_up3, W_c3):
    # L2 input = h natural ordering: feat = c*8 + j   (c<32, j<8)
    r2 = _fused_matrices(
        W_up1, W_c1, 5, 2, 8,
        in_idx=lambda j: np.arange(32) * 8 + j,
        out_idx=lambda x: x * 16 + np.arange(16),
        n_out_cols=256,
    )
    # L3 input ordering: feat = j*16 + c ; output feat = x*8 + o
    r3 = _fused_matrices(
        W_up2, W_c2, 5, 2, 16,
        in_idx=lambda j: j * 16 + np.arange(16),
        out_idx=lambda x: x * 8 + np.arange(8),
        n_out_cols=256,
    )
    # L4 input ordering: feat = j*8 + c ; output col = x*6 + o  (x-major:
    # this makes each 128-col group read only a 128-feat j-window)
    r4 = _fused_matrices(
        W_up3, W_c3, 2, 2, 32,
        in_idx=lambda j: j * 8 + np.arange(8),
        out_idx=lambda x: x * 6 + np.arange(6),
        n_out_cols=384,
    )
    return np.asarray(W_lin, np.float32), r2, r3, r4


def _contribs(p, i, n_in_rows, mats, sy):
    out = []
    for d in (-1, 0, 1):
        if (p, d) in mats and 0 <= i + d < n_in_rows:
            out.append((i + d, mats[(p, d)]))
    return out


def numpy_forward(latent, W_lin, b_lin, r2, r3, r4):
    """Pure-numpy forward through the fused matrices (golden check)."""
    def leaky(x):
        return np.where(x > 0, x, 0.01 * x)

    h = leaky(latent.astype(np.float32) @ W_lin + b_lin)  # [B, 256]
    rows = h[:, None, :]  # [B, 1, 256]
    for (mats, sy, n_in) in ((r2, 5, 1), (r3, 5, 5)):
        nrows = n_in * sy
        out = np.zeros((h.shape[0], nrows, 256), np.float32)
        for y in range(nrows):
            i, p = divmod(y, sy)
            for (src, M) in _contribs(p, i, n_in, mats, sy):
                out[:, y] += rows[:, src] @ M
        rows = leaky(out)
    out = np.zeros((h.shape[0], 50, 384), np.float32)
    for y in range(50):
        i, p = divmod(y, 2)
        for (src, M) in _contribs(p, i, 25, r4, 2):
            out[:, y] += rows[:, src] @ M
    out = np.maximum(out, 0.0)
    # cols are x-major (x*6+o): [B, 50, 64, 6] -> NCHW [B, 6, 50, 64]
    return out.reshape(-1, 50, 64, 6).transpose(0, 3, 1, 2)


# keys in fixed order for weight-tile indexing
R3_KEYS = [(0, -1), (0, 0), (1, 0), (2, 0), (3, 0), (4, 0), (4, 1)]
# L4 k-tile pairs per output phase p (ordered by ascending source row)
R4_PAIRS = (((0, -1), (0, 0)), ((1, 0), (1, 1)))


def _key_contribs(p, i, n_in, keys):
    out = []
    for d in (-1, 0, 1):
        if (p, d) in keys and 0 <= i + d < n_in:
            out.append((i + d, keys.index((p, d))))
    return out


# ---------------------------------------------------------------- bass build
_CACHED = {}


def build_nc():
    nc = bacc.Bacc('TRN2', target_bir_lowering=False, debug=False,
                   num_devices=N_CORES)

    # w1 (cols 0:256) and latent (cols 256:256+BL) share one DMA; row 4 is
    # (b_lin | ones) so the bias rides the contraction for free.
    wlat = nc.declare_dram_parameter("wlat", [5, 256 + BL], F32R, isOutput=False)
    # w2: (y, kc, mc) 128x128 blocks of the 5 R2 row matrices
    w2 = nc.declare_dram_parameter("w2", [128, 5, 2, 2, 128], BF16, isOutput=False)
    # w3: (mat, kc, mc) 128x128 blocks of the 7 R3 matrices (x s3 scale)
    w3 = nc.declare_dram_parameter("w3", [128, 7, 2, 2, 128], BF16, isOutput=False)
    # w4 hi/lo: (p, ktile-mat, g) feat-window x 128-col blocks, e4m3 split
    w4h = nc.declare_dram_parameter("w4h", [128, 2, 2, 3, 128], F8, isOutput=False)
    w4l = nc.declare_dram_parameter("w4l", [128, 2, 2, 3, 128], F8, isOutput=False)
    # out stored (b, y, x*6+o) in bf16 at scale s3*sw4; host descales + casts
    out = nc.declare_dram_parameter("out", [BL, 50, 384], BF16, isOutput=True)

    LR = mybir.ActivationFunctionType.Lrelu
    RELU = mybir.ActivationFunctionType.Relu

    with tile.TileContext(nc) as tc:
        with (
            tc.tile_pool(name="wpool", bufs=1) as wp,
            tc.tile_pool(name="acts", bufs=1) as ap,
            tc.tile_pool(name="lraw", bufs=4) as lp,
            tc.tile_pool(name="ps", bufs=3, space=bass.MemorySpace.PSUM) as pp,
            tc.tile_pool(name="ps3", bufs=1, space=bass.MemorySpace.PSUM) as pp3,
            tc.tile_pool(name="outp", bufs=10) as op,
        ):
            wlat_t = wp.tile([5, 256 + BL], F32R, tag="wlat")
            nc.sync.dma_start(out=wlat_t[:], in_=wlat[:])
            w2_t = wp.tile([128, 5, 2, 2, 128], BF16, tag="w2")
            nc.sync.dma_start(out=w2_t[:], in_=w2[:])
            w3_t = wp.tile([128, 7, 2, 2, 128], BF16, tag="w3")
            nc.sync.dma_start(out=w3_t[:], in_=w3[:])
            w4h_t = wp.tile([128, 2, 2, 3, 128], F8, tag="w4h")
            nc.sync.dma_start(out=w4h_t[:], in_=w4h[:])
            w4l_t = wp.tile([128, 2, 2, 3, 128], F8, tag="w4l")
            nc.sync.dma_start(out=w4l_t[:], in_=w4l[:])

            # x3 hi/lo in fp8, with zeroed pad rows 0 and 26 so L4 edge rows
            # need no special weights; xm* hold the stitched middle window.
            x3h = ap.tile([128, 27, 2, BL], F8, tag="x3h")
            x3l = ap.tile([128, 27, 2, BL], F8, tag="x3l")
            xmh = ap.tile([96, 27, BL], F8, tag="xmh")
            xml = ap.tile([96, 27, BL], F8, tag="xml")
            for t4 in (x3h, x3l):
                nc.gpsimd.memset(t4[:, 0, :, :], 0.0)
                nc.gpsimd.memset(t4[:, 26, :, :], 0.0)
            for t4 in (xmh, xml):
                nc.gpsimd.memset(t4[:, 0, :], 0.0)
                nc.gpsimd.memset(t4[:, 26, :], 0.0)

            # PE p-state warmup: matmuls on a zeroed tile start the frequency
            # ramp while the first weight DMA is still in flight.
            warm = wp.tile([128, 128], BF16, tag="warm")
            nc.vector.memset(warm[:], 0.0)
            for _ in range(16):
                wps = pp.tile([128, 2, BL], F32, tag="ps")
                nc.tensor.matmul(wps[:, 0, 0:128], warm[:], warm[:],
                                 start=True, stop=True)

            # ---- L1: h[256, B] = leaky(W_lin.T @ lat + b)
            x1 = ap.tile([128, 2, BL], BF16, tag="x1")
            ps = pp.tile([128, 2, BL], F32, tag="ps")
            for mc in range(2):
                nc.tensor.matmul(
                    ps[:, mc, :], wlat_t[:, bass.ts(mc, 128)], wlat_t[:, 256:256 + BL],
                    start=True, stop=True,
                )
            for mc in range(2):
                nc.scalar.activation(x1[:, mc, :], ps[:, mc, :], LR, alpha=0.01)

            # ---- L2: 256 -> 1280 (5 rows x 256), input has 1 row
            x2 = ap.tile([128, 5, 2, BL], BF16, tag="x2")

            def emit_l2_row(y):
                ps = pp.tile([128, 2, BL], F32, tag="ps")
                for mc in range(2):
                    for kc in range(2):
                        nc.tensor.matmul(
                            ps[:, mc, :], w2_t[:, y, kc, mc, :], x1[:, kc, :],
                            start=(kc == 0), stop=(kc == 1),
                        )
                nc.scalar.activation(x2[:, y, :, :], ps[:, :, :], LR, alpha=0.01)

            # ---- interleaved L3 (bf16, psum in fp8 domain via s3-scaled w3)
            # and L4 (fp8e4 DoubleRow 3-term).  L3 row y -> x3 hi/lo padded
            # row y+1; after each 5-row group the middle window is stitched
            # and the L4 row-pairs whose 3-row input window is complete run.
            # 6A:5V relu split: DVE also carries the L3 lo-subs, so ACT takes
            # a bit more relu to keep both engines just under the PE cadence.
            relu_cycle = ("A", "V") * 5 + ("A",)
            n_relu = 0
            ob = [None] * 4
            p0 = [0] * 4

            def emit_l3_row(y):
                i, p = divmod(y, 5)
                cs = _key_contribs(p, i, 5, R3_KEYS)
                ps = pp3.tile([128, 2, BL], F32, tag="ps3")
                for mc in range(2):
                    n, tot = 0, len(cs) * 2
                    for (src, mi) in cs:
                        for kc in range(2):
                            nc.tensor.matmul(
                                ps[:, mc, :], w3_t[:, mi, kc, mc, :],
                                x2[:, src, kc, :],
                                start=(n == 0), stop=(n == tot - 1),
                            )
                            n += 1
                # hi = e4m3(leaky(ps) * 15/16): floor-shift so the true
                # positive-lane residual ps - hi lands in [0, ~2 ulp).
                nc.scalar.activation(x3h[:, y + 1, :, :], ps[:, :, :], LR,
                                     alpha=0.01, scale=15.0 / 16.0)
                lr = lp.tile([128, 2, BL], F8, tag="lr", name=f"lr_{y}")
                nc.vector.tensor_tensor(out=lr[:], in0=ps[:, :, :],
                                        in1=x3h[:, y + 1, :, :],
                                        op=mybir.AluOpType.subtract)
                # gate the negative-lane junk (strictly negative there)
                nc.gpsimd.tensor_scalar_max(x3l[:, y + 1, :, :], lr[:], 0.0)

            def emit_stitch(y0, n):
                # rows y0..y0+n-1 (padded +1): feats 80..176
                for (src3, dst3) in ((x3h, xmh), (x3l, xml)):
                    nc.sync.dma_start(
                        out=dst3[0:48, y0 + 1:y0 + 1 + n, :],
                        in_=src3[80:128, y0 + 1:y0 + 1 + n, 0, :])
                    nc.sync.dma_start(
                        out=dst3[48:96, y0 + 1:y0 + 1 + n, :],
                        in_=src3[0:48, y0 + 1:y0 + 1 + n, 1, :])

            def emit_l4_pair(pr):
                nonlocal n_relu
                for bb in range(4):
                    bs = bass.ts(bb, 128)
                    if ob[bb] is None:
                        ob[bb] = op.tile([128, 4, 384], BF16, tag="ob",
                                         name=f"ob_{bb}_{pr}")
                        p0[bb] = pr
                    ps = pp.tile([128, 2, 512], F32, tag="ps")
                    for yy in range(2):
                        base = pr + yy  # padded start row of the k-tile pair
                        for g in range(3):
                            if g == 1:
                                ah = xmh[:, base:base + 2, bs]
                                al = xml[:, base:base + 2, bs]
                                wh = w4h_t[0:96, yy, :, g, :]
                                wl = w4l_t[0:96, yy, :, g, :]
                            else:
                                c = 0 if g == 0 else 1
                                ah = x3h[:, base:base + 2, c, bs]
                                al = x3l[:, base:base + 2, c, bs]
                                wh = w4h_t[:, yy, :, g, :]
                                wl = w4l_t[:, yy, :, g, :]
                            dst = ps[:, yy, bass.ts(g, 128)]
                            nc.tensor.matmul(dst, ah, wh, start=True,
                                             stop=False, perf_mode=DR)
                            nc.tensor.matmul(dst, al, wh, start=False,
                                             stop=False, perf_mode=DR)
                            nc.tensor.matmul(dst, ah, wl, start=False,
                                             stop=True, perf_mode=DR)
                    if bb == 3 and pr == 24:
                        eng = "V"  # keep the final relu off the busier ACT
                    else:
                        eng = relu_cycle[n_relu % len(relu_cycle)]
                        n_relu += 1
                    r0 = 2 * (pr - p0[bb])
                    dst = ob[bb][:, r0:r0 + 2, :]
                    src_ap = ps[:, :, 0:384]
                    if eng == "A":
                        nc.scalar.activation(dst, src_ap, RELU)
                    else:
                        nc.vector.tensor_scalar_max(dst, src_ap, 0.0)
                    if pr % 2 == bb % 2 or pr >= 23:
                        nrow = 2 * (pr - p0[bb]) + 2
                        nc.sync.dma_start(
                            out=out[bs, 2 * p0[bb]:2 * p0[bb] + nrow, :],
                            in_=ob[bb][:, 0:nrow, :],
                        )
                        ob[bb] = None

            # Row-granular software pipeline: L4 pair pr is emitted ~2 rows
            # after its xm stitch, so its x3/xm inputs went through the full
            # ACT->DVE->Pool->stitch chain before the PE gets there and the
            # PE never stalls (stalls also reset the p-state ramp, costing
            # 2x cycles for 3us after each one).  Stitches go every 5 rows
            # except the last group, split finer so fewer pairs trail.
            stitch_plan = {4: (0, 5), 9: (5, 5), 14: (10, 5), 19: (15, 5),
                           24: (20, 5)}
            LAG = 8
            # L3 rows 0-3 read only x2 row 0, so they interleave between the
            # L2 rows to fill L2's ACT-latency chain (deps only cover writes
            # already emitted, so an early L3 row does not wait on later L2
            # rows).
            emit_l2_row(0)
            for y in range(4):
                emit_l3_row(y)
                emit_l2_row(y + 1)
            for y in range(4, 25):
                emit_l3_row(y)
                if y in stitch_plan:
                    emit_stitch(*stitch_plan[y])
                if y >= LAG:
                    emit_l4_pair(y - LAG)
            for pr in range(25 - LAG, 25):
                emit_l4_pair(pr)
    nc.compile()
    return nc


# ---------------------------------------------------------------- entry
def kernel(**inputs):
    latent = np.asarray(inputs["latent_vector"], np.float32)
    W_lin, r2, r3, r4 = build_host_matrices(
        inputs["W_lin"], inputs["W_up1"], inputs["W_c1"],
        inputs["W_up2"], inputs["W_c2"], inputs["W_up3"], inputs["W_c3"],
    )
    b_lin = np.asarray(inputs["b_lin"], np.float32)

    if "nc" not in _CACHED:
        _CACHED["nc"] = build_nc()
    nc = _CACHED["nc"]

    # fp8 scales: x3 absmax from a subset forward (7x headroom to e4m3's
    # 448 saturation), w4 absmax exact.  Both powers of 2 so the host-side
    # descale of the output is lossless.
    def leaky(x):
        return np.where(x > 0, x, 0.01 * x)

    sub = latent[:256]
    h = leaky(sub @ W_lin + b_lin)
    rows = h[:, None, :]
    for (mats, sy, n_in) in ((r2, 5, 1), (r3, 5, 5)):
        nrows = n_in * sy
        o = np.zeros((sub.shape[0], nrows, 256), np.float32)
        for y in range(nrows):
            i, p = divmod(y, sy)
            for (src, M) in _contribs(p, i, n_in, mats, sy):
                o[:, y] += rows[:, src] @ M
        rows = leaky(o)
    x3max = float(np.abs(rows).max())
    w4max = max(float(np.abs(M).max()) for M in r4.values())
    s3 = 2.0 ** np.floor(np.log2(64.0 / x3max))
    sw4 = 2.0 ** np.floor(np.log2(64.0 / w4max))
    _CACHED["descale"] = 1.0 / (s3 * sw4)

    # w2/w3: [K=128, n, kc, mc, 128] layouts (w3 carries the s3 scale)
    def pack_blocks(mats_list, scale=1.0):
        n = len(mats_list)
        t = np.zeros((128, n, 2, 2, 128), np.float32)
        for mi, M in enumerate(mats_list):
            Ms = M * scale
            for kc in range(2):
                for mc in range(2):
                    t[:, mi, kc, mc, :] = Ms[kc * 128:(kc + 1) * 128,
                                             mc * 128:(mc + 1) * 128]
        return np.ascontiguousarray(t.astype(NPBF))

    w2_host = pack_blocks([r2[(p, 0)] for p in range(5)])
    w3_host = pack_blocks([r3[k] for k in R3_KEYS], scale=s3)

    # w4 hi/lo: [128, p, ktile, g, 128]; k-tile t of phase p is R4_PAIRS[p][t]
    # windowed per group g (rows L4_WIN[g], cols 128g..128(g+1)), e4m3-split
    # of sw4-scaled values.
    w4h_host = np.zeros((128, 2, 2, 3, 128), NPF8)
    w4l_host = np.zeros((128, 2, 2, 3, 128), NPF8)
    for p in range(2):
        for t, key in enumerate(R4_PAIRS[p]):
            M = r4[key] * sw4
            for g, (r0, r1) in enumerate(L4_WIN):
                blk = M[r0:r1, g * 128:(g + 1) * 128]
                rest = M[:, g * 128:(g + 1) * 128].copy()
                rest[r0:r1] = 0.0
                assert np.all(rest == 0.0), f"L4 window violated {key} g{g}"
                hi = blk.astype(NPF8)
                lo = (blk - hi.astype(np.float32)).astype(NPF8)
                w4h_host[:r1 - r0, p, t, g, :] = hi
                w4l_host[:r1 - r0, p, t, g, :] = lo
    w4h_host = np.ascontiguousarray(w4h_host)
    w4l_host = np.ascontiguousarray(w4l_host)

    base = {"w2": w2_host, "w3": w3_host, "w4h": w4h_host, "w4l": w4l_host}

    w1b = np.concatenate([W_lin, b_lin[None, :]], axis=0)  # [5, 256]
    in_maps = []
    for c in range(N_CORES):
        sh = latent[c * BL:(c + 1) * BL]
        lat1 = np.concatenate(
            [sh.T, np.ones((1, BL), np.float32)], axis=0)  # [5, BL]
        wlat = np.concatenate([w1b, lat1], axis=1)
        in_maps.append({**base,
                        "wlat": np.ascontiguousarray(wlat)})

    _CACHED["maps"] = in_maps
    res = run_bass_kernel_spmd(nc, in_maps, list(range(N_CORES)))
    descale = _CACHED["descale"]
    outs = [
        (np.asarray(r["out"], NPBF).astype(np.float32) * descale)
        .reshape(BL, 50, 64, 6).transpose(0, 3, 1, 2)
        for r in res.results
    ]
    return np.ascontiguousarray(np.concatenate(outs, axis=0))


if __name__ == "__main__":
    rng = np.random.default_rng(0)
    fake = {
        "latent_vector": rng.standard_normal((B, 4)).astype(np.float32),
        "W_lin": rng.standard_normal((4, 256)).astype(np.float32) * 0.5,
        "b_lin": np.zeros(256, np.float32),
        "W_up1": rng.standard_normal((5, 2, 32, 32)).astype(np.float32) * 0.1,
        "W_c1": rng.standard_normal((3, 3, 32, 16)).astype(np.float32) * 0.1,
        "W_up2": rng.standard_normal((5, 2, 16, 16)).astype(np.float32) * 0.1,
        "W_c2": rng.standard_normal((3, 3, 16, 8)).astype(np.float32) * 0.1,
        "W_up3": rng.standard_normal((2, 2, 8, 8)).astype(np.float32) * 0.1,
        "W_c3": rng.standard_normal((3, 3, 8, 6)).astype(np.float32) * 0.1,
    }
    o = kernel(**fake)
    print("kernel out", o.shape, o.dtype)


# revision 21
# speedup vs baseline: 1.2395x; 1.2395x over previous
"""Trainium2 Bass kernel for nn_BetaVAEMark10Decoder.

Network (per sample): latent(4) -> Linear(256)+leaky -> reshape (1,8,32)
 -> convT(5,2)s(5,2) -> conv3x3 SAME +leaky   (5,16,16)
 -> convT(5,2)s(5,2) -> conv3x3 SAME +leaky   (25,32,8)
 -> convT(2,2)s(2,2) -> conv3x3 SAME +relu    (50,64,6)  -> NCHW out.

Each convT(k=s) + 3x3 conv pair composes into one exact linear map that is
block-banded over rows: output row y reads input rows i+d through per-phase
matrices R[p, d].  Everything becomes dense matmuls on 128-chunks.

v2: the final layer (L4, 2/3 of all PE cycles) runs as fp8e4m3 DoubleRow
matmuls at 0.5 cycles/row with the two row-contributions packed into the two
DR k-tiles, so one DR replaces two bf16 matmuls (4x MACs/cycle).  Precision
is restored by a 3-term compensated product
    x3 @ W  ~=  hi@Whi + lo@Whi + hi@Wlo
where (Whi, Wlo) is an exact host-side e4m3 split of the (scaled) weights
and (hi, lo) is an on-device split of x3 produced in 2 1/3 engine passes:
  ACT:  hi = e4m3(leaky(ps) * 15/16)     (psum is pre-scaled into fp8 domain
                                          by folding s3 into the bf16 w3)
  DVE:  lo_raw = ps - hi  -> e4m3        (on ps<0 lanes this is ~0.99*ps < 0)
  Pool: lo = max(lo_raw, 0)              (gates off the negative-lane junk;
                                          the 15/16 "floor shift" makes the
                                          true positive-lane residual >= 0)
The leaky'd negative lanes are ~1% magnitude, so hi alone carries them.
Measured end-to-end error 0.63% vs the fp32 reference (bf16 baseline 0.58%).

L4 row-pair edges use two zeroed pad rows in x3hi/x3lo instead of special
edge weights, so every output row is exactly one (pair, 3-group, 3-term)
DR set.  L3 rows and L4 row-pairs are emitted interleaved so the 19.7MB
output DMA (54.6us at 360GB/s, the largest single resource) spreads over
the whole kernel instead of piling into an L4-only tail.

Other levers kept from v1: bias folded into the L1 contraction as a 5th
ones-row, PE p-state warm-up matmuls under the first weight DMA, 8-bank
psum pool of [128,2,512] tiles, relu split across ACT and DVE (GPSIMD
cannot read PSUM) with the split ratio rebalanced for the DVE's new L3-sub
load, bf16 output (upcast + pow2 descale on host) flushed staggered by
batch-block parity.

Sharding: pure data parallel, batch 4096 -> 8 cores x 512.
"""

import sys

import numpy as np

sys.path.insert(0, "/opt/trn_rl_repo")

import ml_dtypes  # noqa: E402

import concourse.bass as bass  # noqa: E402
import concourse.bacc as bacc  # noqa: E402
import concourse.mybir as mybir  # noqa: E402
from concourse import tile  # noqa: E402
from concourse.bass_utils import run_bass_kernel_spmd  # noqa: E402

N_CORES = 8
B = 4096
BL = B // N_CORES  # 512 per core
F32 = mybir.dt.float32
F32R = mybir.dt.float32r
BF16 = mybir.dt.bfloat16
F8 = mybir.dt.float8e4
NPBF = ml_dtypes.bfloat16
NPF8 = (ml_dtypes.float8_e4m3fn if hasattr(ml_dtypes, "float8_e4m3fn")
        else ml_dtypes.float8_e4m3)
DR = mybir.MatmulPerfMode.DoubleRow

# L4 feature windows per column group: group g of an output row (cols
# 128g..128(g+1), x-major) only reads input feats within these windows.
L4_WIN = ((0, 128), (80, 176), (128, 256))


# ---------------------------------------------------------------- host math
def _fused_matrices(Wup, Wc, sy, sx, Win, in_idx, out_idx, n_out_cols):
    """Compose convT(k=s=(sy,sx)) with 3x3 SAME conv into per-phase row
    matrices.  Returns {(p, delta): M} where out row y (p = y%sy, i = y//sy)
    accumulates  in_row[i+delta] @ M[(p, delta)]  over available deltas.
    x-edge clipping is baked into M; y-edge clipping == skipping absent rows.
    """
    Wup = np.asarray(Wup, np.float32)
    Wc = np.asarray(Wc, np.float32)
    Cin = Wup.shape[2]
    Wout = Win * sx
    mats = {}
    for p in range(sy):
        deltas = {0}
        if p == 0:
            deltas.add(-1)
        if p == sy - 1:
            deltas.add(1)
        for d in sorted(deltas):
            M = np.zeros((Win * Cin, n_out_cols), np.float32)
            y = sy + p  # representative interior row
            i_t = y // sy + d
            nz = False
            for dy in (-1, 0, 1):
                yp = y + dy
                if yp // sy != i_t:
                    continue
                py = yp % sy
                for x in range(Wout):
                    for dx in (-1, 0, 1):
                        xp = x + dx
                        if xp < 0 or xp >= Wout:
                            continue
                        j, qx = divmod(xp, sx)
                        # conv_transpose (transpose_kernel=False) applies the
                        # spatially mirrored kernel per phase
                        CC = Wup[sy - 1 - py, sx - 1 - qx] @ Wc[dy + 1, dx + 1]
                        M[np.ix_(in_idx(j), out_idx(x))] += CC
                        nz = True
            if nz:
                mats[(p, d)] = M
    return mats


def build_host_matrices(W_lin, W_up1, W_c1, W_up2, W_c2, W_up3, W_c3):
    # L2 input = h natural ordering: feat = c*8 + j   (c<32, j<8)
    r2 = _fused_matrices(
        W_up1, W_c1, 5, 2, 8,
        in_idx=lambda j: np.arange(32) * 8 + j,
        out_idx=lambda x: x * 16 + np.arange(16),
        n_out_cols=256,
    )
    # L3 input ordering: feat = j*16 + c ; output feat = x*8 + o
    r3 = _fused_matrices(
        W_up2, W_c2, 5, 2, 16,
        in_idx=lambda j: j * 16 + np.arange(16),
        out_idx=lambda x: x * 8 + np.arange(8),
        n_out_cols=256,
    )
    # L4 input ordering: feat = j*8 + c ; output col = x*6 + o  (x-major:
    # this makes each 128-col group read only a 128-feat j-window)
    r4 = _fused_matrices(
        W_up3, W_c3, 2, 2, 32,
        in_idx=lambda j: j * 8 + np.arange(8),
        out_idx=lambda x: x * 6 + np.arange(6),
        n_out_cols=384,
    )
    return np.asarray(W_lin, np.float32), r2, r3, r4


def _contribs(p, i, n_in_rows, mats, sy):
    out = []
    for d in (-1, 0, 1):
        if (p, d) in mats and 0 <= i + d < n_in_rows:
            out.append((i + d, mats[(p, d)]))
    return out


def numpy_forward(latent, W_lin, b_lin, r2, r3, r4):
    """Pure-numpy forward through the fused matrices (golden check)."""
    def leaky(x):
        return np.where(x > 0, x, 0.01 * x)

    h = leaky(latent.astype(np.float32) @ W_lin + b_lin)  # [B, 256]
    rows = h[:, None, :]  # [B, 1, 256]
    for (mats, sy, n_in) in ((r2, 5, 1), (r3, 5, 5)):
        nrows = n_in * sy
        out = np.zeros((h.shape[0], nrows, 256), np.float32)
        for y in range(nrows):
            i, p = divmod(y, sy)
            for (src, M) in _contribs(p, i, n_in, mats, sy):
                out[:, y] += rows[:, src] @ M
        rows = leaky(out)
    out = np.zeros((h.shape[0], 50, 384), np.float32)
    for y in range(50):
        i, p = divmod(y, 2)
        for (src, M) in _contribs(p, i, 25, r4, 2):
            out[:, y] += rows[:, src] @ M
    out = np.maximum(out, 0.0)
    # cols are x-major (x*6+o): [B, 50, 64, 6] -> NCHW [B, 6, 50, 64]
    return out.reshape(-1, 50, 64, 6).transpose(0, 3, 1, 2)


# keys in fixed order for weight-tile indexing
R3_KEYS = [(0, -1), (0, 0), (1, 0), (2, 0), (3, 0), (4, 0), (4, 1)]
# L4 k-tile pairs per output phase p (ordered by ascending source row)
R4_PAIRS = (((0, -1), (0, 0)), ((1, 0), (1, 1)))


def _key_contribs(p, i, n_in, keys):
    out = []
    for d in (-1, 0, 1):
        if (p, d) in keys and 0 <= i + d < n_in:
            out.append((i + d, keys.index((p, d))))
    return out


# ---------------------------------------------------------------- bass build
_CACHED = {}


def build_nc():
    nc = bacc.Bacc('TRN2', target_bir_lowering=False, debug=False,
                   num_devices=N_CORES)

    # w1 (cols 0:256) and latent (cols 256:256+BL) share one DMA; row 4 is
    # (b_lin | ones) so the bias rides the contraction for free.
    wlat = nc.declare_dram_parameter("wlat", [5, 256 + BL], F32R, isOutput=False)
    # w2: (y, kc, mc) 128x128 blocks of the 5 R2 row matrices
    w2 = nc.declare_dram_parameter("w2", [128, 5, 2, 2, 128], BF16, isOutput=False)
    # w3: (mat, kc, mc) 128x128 blocks of the 7 R3 matrices (x s3 scale)
    w3 = nc.declare_dram_parameter("w3", [128, 7, 2, 2, 128], BF16, isOutput=False)
    # w4 hi/lo: (p, ktile-mat, g) feat-window x 128-col blocks, e4m3 split
    w4h = nc.declare_dram_parameter("w4h", [128, 2, 2, 3, 128], F8, isOutput=False)
    w4l = nc.declare_dram_parameter("w4l", [128, 2, 2, 3, 128], F8, isOutput=False)
    # out stored (b, y, x*6+o) in bf16 at scale s3*sw4; host descales + casts
    out = nc.declare_dram_parameter("out", [BL, 50, 384], BF16, isOutput=True)

    LR = mybir.ActivationFunctionType.Lrelu
    RELU = mybir.ActivationFunctionType.Relu

    with tile.TileContext(nc) as tc:
        with (
            tc.tile_pool(name="wpool", bufs=1) as wp,
            tc.tile_pool(name="acts", bufs=1) as ap,
            tc.tile_pool(name="lraw", bufs=4) as lp,
            tc.tile_pool(name="ps", bufs=4, space=bass.MemorySpace.PSUM) as pp,
            tc.tile_pool(name="outp", bufs=10) as op,
        ):
            wlat_t = wp.tile([5, 256 + BL], F32R, tag="wlat")
            nc.sync.dma_start(out=wlat_t[:], in_=wlat[:])
            w2_t = wp.tile([128, 5, 2, 2, 128], BF16, tag="w2")
            nc.sync.dma_start(out=w2_t[:], in_=w2[:])
            w3_t = wp.tile([128, 7, 2, 2, 128], BF16, tag="w3")
            nc.sync.dma_start(out=w3_t[:], in_=w3[:])
            w4h_t = wp.tile([128, 2, 2, 3, 128], F8, tag="w4h")
            nc.sync.dma_start(out=w4h_t[:], in_=w4h[:])
            w4l_t = wp.tile([128, 2, 2, 3, 128], F8, tag="w4l")
            nc.sync.dma_start(out=w4l_t[:], in_=w4l[:])

            # x3 hi/lo in fp8, with zeroed pad rows 0 and 26 so L4 edge rows
            # need no special weights; xm* hold the stitched middle window.
            x3h = ap.tile([128, 27, 2, BL], F8, tag="x3h")
            x3l = ap.tile([128, 27, 2, BL], F8, tag="x3l")
            xmh = ap.tile([96, 27, BL], F8, tag="xmh")
            xml = ap.tile([96, 27, BL], F8, tag="xml")
            for t4 in (x3h, x3l):
                nc.gpsimd.memset(t4[:, 0, :, :], 0.0)
                nc.gpsimd.memset(t4[:, 26, :, :], 0.0)
            for t4 in (xmh, xml):
                nc.gpsimd.memset(t4[:, 0, :], 0.0)
                nc.gpsimd.memset(t4[:, 26, :], 0.0)

            # PE p-state warmup: matmuls on a zeroed tile start the frequency
            # ramp while the first weight DMA is still in flight.
            warm = wp.tile([128, 128], BF16, tag="warm")
            nc.vector.memset(warm[:], 0.0)
            for _ in range(16):
                wps = pp.tile([128, 2, BL], F32, tag="ps")
                nc.tensor.matmul(wps[:, 0, 0:128], warm[:], warm[:],
                                 start=True, stop=True)

            # ---- L1: h[256, B] = leaky(W_lin.T @ lat + b)
            x1 = ap.tile([128, 2, BL], BF16, tag="x1")
            ps = pp.tile([128, 2, BL], F32, tag="ps")
            for mc in range(2):
                nc.tensor.matmul(
                    ps[:, mc, :], wlat_t[:, bass.ts(mc, 128)], wlat_t[:, 256:256 + BL],
                    start=True, stop=True,
                )
            for mc in range(2):
                nc.scalar.activation(x1[:, mc, :], ps[:, mc, :], LR, alpha=0.01)

            # ---- L2: 256 -> 1280 (5 rows x 256), input has 1 row
            x2 = ap.tile([128, 5, 2, BL], BF16, tag="x2")

            def emit_l2_row(y):
                ps = pp.tile([128, 2, BL], F32, tag="ps")
                for mc in range(2):
                    for kc in range(2):
                        nc.tensor.matmul(
                            ps[:, mc, :], w2_t[:, y, kc, mc, :], x1[:, kc, :],
                            start=(kc == 0), stop=(kc == 1),
                        )
                nc.scalar.activation(x2[:, y, :, :], ps[:, :, :], LR, alpha=0.01)

            # ---- interleaved L3 (bf16, psum in fp8 domain via s3-scaled w3)
            # and L4 (fp8e4 DoubleRow 3-term).  L3 row y -> x3 hi/lo padded
            # row y+1; after each 5-row group the middle window is stitched
            # and the L4 row-pairs whose 3-row input window is complete run.
            # 6A:5V relu split: DVE also carries the L3 lo-subs, so ACT takes
            # a bit more relu to keep both engines just under the PE cadence.
            relu_cycle = ("A", "V") * 5 + ("A",)
            n_relu = 0
            ob = [None] * 4
            p0 = [0] * 4

            def emit_l3_row(y):
                i, p = divmod(y, 5)
                cs = _key_contribs(p, i, 5, R3_KEYS)
                ps = pp.tile([128, 2, BL], F32, tag="ps")
                for mc in range(2):
                    n, tot = 0, len(cs) * 2
                    for (src, mi) in cs:
                        for kc in range(2):
                            nc.tensor.matmul(
                                ps[:, mc, :], w3_t[:, mi, kc, mc, :],
                                x2[:, src, kc, :],
                                start=(n == 0), stop=(n == tot - 1),
                            )
                            n += 1
                # hi = e4m3(leaky(ps) * 15/16): floor-shift so the true
                # positive-lane residual ps - hi lands in [0, ~2 ulp).
                nc.scalar.activation(x3h[:, y + 1, :, :], ps[:, :, :], LR,
                                     alpha=0.01, scale=15.0 / 16.0)
                lr = lp.tile([128, 2, BL], F8, tag="lr", name=f"lr_{y}")
                nc.vector.tensor_tensor(out=lr[:], in0=ps[:, :, :],
                                        in1=x3h[:, y + 1, :, :],
                                        op=mybir.AluOpType.subtract)
                # gate the negative-lane junk (strictly negative there)
                nc.gpsimd.tensor_scalar_max(x3l[:, y + 1, :, :], lr[:], 0.0)

            def emit_stitch(y0, n):
                # rows y0..y0+n-1 (padded +1): feats 80..176
                for (src3, dst3) in ((x3h, xmh), (x3l, xml)):
                    nc.sync.dma_start(
                        out=dst3[0:48, y0 + 1:y0 + 1 + n, :],
                        in_=src3[80:128, y0 + 1:y0 + 1 + n, 0, :])
                    nc.sync.dma_start(
                        out=dst3[48:96, y0 + 1:y0 + 1 + n, :],
                        in_=src3[0:48, y0 + 1:y0 + 1 + n, 1, :])

            def emit_l4_pair(pr):
                nonlocal n_relu
                for bb in range(4):
                    bs = bass.ts(bb, 128)
                    if ob[bb] is None:
                        ob[bb] = op.tile([128, 4, 384], BF16, tag="ob",
                                         name=f"ob_{bb}_{pr}")
                        p0[bb] = pr
                    ps = pp.tile([128, 2, 512], F32, tag="ps")
                    for yy in range(2):
                        base = pr + yy  # padded start row of the k-tile pair
                        for g in range(3):
                            if g == 1:
                                ah = xmh[:, base:base + 2, bs]
                                al = xml[:, base:base + 2, bs]
                                wh = w4h_t[0:96, yy, :, g, :]
                                wl = w4l_t[0:96, yy, :, g, :]
                            else:
                                c = 0 if g == 0 else 1
                                ah = x3h[:, base:base + 2, c, bs]
                                al = x3l[:, base:base + 2, c, bs]
                                wh = w4h_t[:, yy, :, g, :]
                                wl = w4l_t[:, yy, :, g, :]
                            dst = ps[:, yy, bass.ts(g, 128)]
                            nc.tensor.matmul(dst, ah, wh, start=True,
                                             stop=False, perf_mode=DR)
                            nc.tensor.matmul(dst, al, wh, start=False,
                                             stop=False, perf_mode=DR)
                            nc.tensor.matmul(dst, ah, wl, start=False,
                                             stop=True, perf_mode=DR)
                    if bb == 3 and pr == 24:
                        eng = "V"  # keep the final relu off the busier ACT
                    else:
                        eng = relu_cycle[n_relu % len(relu_cycle)]
                        n_relu += 1
                    r0 = 2 * (pr - p0[bb])
                    dst = ob[bb][:, r0:r0 + 2, :]
                    src_ap = ps[:, :, 0:384]
                    if eng == "A":
                        nc.scalar.activation(dst, src_ap, RELU)
                    else:
                        nc.vector.tensor_scalar_max(dst, src_ap, 0.0)
                    if pr % 2 == bb % 2 or pr >= 23:
                        nrow = 2 * (pr - p0[bb]) + 2
                        nc.sync.dma_start(
                            out=out[bs, 2 * p0[bb]:2 * p0[bb] + nrow, :],
                            in_=ob[bb][:, 0:nrow, :],
                        )
                        ob[bb] = None

            # Row-granular software pipeline: L4 pair pr is emitted ~2 rows
            # after its xm stitch, so its x3/xm inputs went through the full
            # ACT->DVE->Pool->stitch chain before the PE gets there and the
            # PE never stalls (stalls also reset the p-state ramp, costing
            # 2x cycles for 3us after each one).  Stitches go every 5 rows
            # except the last group, split finer so fewer pairs trail.
            stitch_plan = {4: (0, 5), 9: (5, 5), 14: (10, 5), 19: (15, 5),
                           24: (20, 5)}
            LAG = 8
            # L3 rows 0-3 read only x2 row 0, so they interleave between the
            # L2 rows to fill L2's ACT-latency chain (deps only cover writes
            # already emitted, so an early L3 row does not wait on later L2
            # rows).
            emit_l2_row(0)
            for y in range(4):
                emit_l3_row(y)
                emit_l2_row(y + 1)
            for y in range(4, 25):
                emit_l3_row(y)
                if y in stitch_plan:
                    emit_stitch(*stitch_plan[y])
                if y >= LAG:
                    emit_l4_pair(y - LAG)
            for pr in range(25 - LAG, 25):
                emit_l4_pair(pr)
    nc.compile()
    return nc


# ---------------------------------------------------------------- entry
def kernel(**inputs):
    latent = np.asarray(inputs["latent_vector"], np.float32)
    W_lin, r2, r3, r4 = build_host_matrices(
        inputs["W_lin"], inputs["W_up1"], inputs["W_c1"],
        inputs["W_up2"], inputs["W_c2"], inputs["W_up3"], inputs["W_c3"],
    )
    b_lin = np.asarray(inputs["b_lin"], np.float32)

    if "nc" not in _CACHED:
        _CACHED["nc"] = build_nc()
    nc = _CACHED["nc"]

    # fp8 scales: x3 absmax from a subset forward (7x headroom to e4m3's
    # 448 saturation), w4 absmax exact.  Both powers of 2 so the host-side
    # descale of the output is lossless.
    def leaky(x):
        return np.where(x > 0, x, 0.01 * x)

    sub = latent[:256]
    h = leaky(sub @ W_lin + b_lin)
    rows = h[:, None, :]
    for (mats, sy, n_in) in ((r2, 5, 1), (r3, 5, 5)):
        nrows = n_in * sy
        o = np.zeros((sub.shape[0], nrows, 256), np.float32)
        for y in range(nrows):
            i, p = divmod(y, sy)
            for (src, M) in _contribs(p, i, n_in, mats, sy):
                o[:, y] += rows[:, src] @ M
        rows = leaky(o)
    x3max = float(np.abs(rows).max())
    w4max = max(float(np.abs(M).max()) for M in r4.values())
    s3 = 2.0 ** np.floor(np.log2(64.0 / x3max))
    sw4 = 2.0 ** np.floor(np.log2(64.0 / w4max))
    _CACHED["descale"] = 1.0 / (s3 * sw4)

    # w2/w3: [K=128, n, kc, mc, 128] layouts (w3 carries the s3 scale)
    def pack_blocks(mats_list, scale=1.0):
        n = len(mats_list)
        t = np.zeros((128, n, 2, 2, 128), np.float32)
        for mi, M in enumerate(mats_list):
            Ms = M * scale
            for kc in range(2):
                for mc in range(2):
                    t[:, mi, kc, mc, :] = Ms[kc * 128:(kc + 1) * 128,
                                             mc * 128:(mc + 1) * 128]
        return np.ascontiguousarray(t.astype(NPBF))

    w2_host = pack_blocks([r2[(p, 0)] for p in range(5)])
    w3_host = pack_blocks([r3[k] for k in R3_KEYS], scale=s3)

    # w4 hi/lo: [128, p, ktile, g, 128]; k-tile t of phase p is R4_PAIRS[p][t]
    # windowed per group g (rows L4_WIN[g], cols 128g..128(g+1)), e4m3-split
    # of sw4-scaled values.
    w4h_host = np.zeros((128, 2, 2, 3, 128), NPF8)
    w4l_host = np.zeros((128, 2, 2, 3, 128), NPF8)
    for p in range(2):
        for t, key in enumerate(R4_PAIRS[p]):
            M = r4[key] * sw4
            for g, (r0, r1) in enumerate(L4_WIN):
                blk = M[r0:r1, g * 128:(g + 1) * 128]
                rest = M[:, g * 128:(g + 1) * 128].copy()
                rest[r0:r1] = 0.0
                assert np.all(rest == 0.0), f"L4 window violated {key} g{g}"
                hi = blk.astype(NPF8)
                lo = (blk - hi.astype(np.float32)).astype(NPF8)
                w4h_host[:r1 - r0, p, t, g, :] = hi
                w4l_host[:r1 - r0, p, t, g, :] = lo
    w4h_host = np.ascontiguousarray(w4h_host)
    w4l_host = np.ascontiguousarray(w4l_host)

    base = {"w2": w2_host, "w3": w3_host, "w4h": w4h_host, "w4l": w4l_host}

    w1b = np.concatenate([W_lin, b_lin[None, :]], axis=0)  # [5, 256]
    in_maps = []
    for c in range(N_CORES):
        sh = latent[c * BL:(c + 1) * BL]
        lat1 = np.concatenate(
            [sh.T, np.ones((1, BL), np.float32)], axis=0)  # [5, BL]
        wlat = np.concatenate([w1b, lat1], axis=1)
        in_maps.append({**base,
                        "wlat": np.ascontiguousarray(wlat)})

    _CACHED["maps"] = in_maps
    res = run_bass_kernel_spmd(nc, in_maps, list(range(N_CORES)))
    descale = _CACHED["descale"]
    outs = [
        (np.asarray(r["out"], NPBF).astype(np.float32) * descale)
        .reshape(BL, 50, 64, 6).transpose(0, 3, 1, 2)
        for r in res.results
    ]
    return np.ascontiguousarray(np.concatenate(outs, axis=0))


if __name__ == "__main__":
    rng = np.random.default_rng(0)
    fake = {
        "latent_vector": rng.standard_normal((B, 4)).astype(np.float32),
        "W_lin": rng.standard_normal((4, 256)).astype(np.float32) * 0.5,
        "b_lin": np.zeros(256, np.float32),
        "W_up1": rng.standard_normal((5, 2, 32, 32)).astype(np.float32) * 0.1,
        "W_c1": rng.standard_normal((3, 3, 32, 16)).astype(np.float32) * 0.1,
        "W_up2": rng.standard_normal((5, 2, 16, 16)).astype(np.float32) * 0.1,
        "W_c2": rng.standard_normal((3, 3, 16, 8)).astype(np.float32) * 0.1,
        "W_up3": rng.standard_normal((2, 2, 8, 8)).astype(np.float32) * 0.1,
        "W_c3": rng.standard_normal((3, 3, 8, 6)).astype(np.float32) * 0.1,
    }
    o = kernel(**fake)
    print("kernel out", o.shape, o.dtype)


# revision 23
# speedup vs baseline: 1.2962x; 1.0457x over previous
"""Trainium2 Bass kernel for nn_BetaVAEMark10Decoder.

Network (per sample): latent(4) -> Linear(256)+leaky -> reshape (1,8,32)
 -> convT(5,2)s(5,2) -> conv3x3 SAME +leaky   (5,16,16)
 -> convT(5,2)s(5,2) -> conv3x3 SAME +leaky   (25,32,8)
 -> convT(2,2)s(2,2) -> conv3x3 SAME +relu    (50,64,6)  -> NCHW out.

Each convT(k=s) + 3x3 conv pair composes into one exact linear map that is
block-banded over rows: output row y reads input rows i+d through per-phase
matrices R[p, d].  Everything becomes dense matmuls on 128-chunks.

This version runs the whole stack in bf16 (1 cycle/row on the PE for any N,
vs fp32r's 4x penalty under N=256) and exploits the *x*-banded structure of
the final layer: with x-major output ordering (col = x*6 + o), each 128-col
group of an output row only reads a feature window of the input row
(j-window * 8 ch): feats 0..128 / 80..176 / 128..256.  The outer windows are
the two natural 128-chunks of x3; the middle one is stitched by a cheap
SBUF-to-SBUF DMA.  L4 then needs only 3 matmuls of N=128 per (row-contrib,
batch-block): 150k PE cycles instead of the 301k a dense 2-chunk
contraction costs.  The kernel is PE-bound at ~92% occupancy (~97us busy).

Other levers: bias folded into the L1 contraction as a 5th ones-row (no
bias DMA/path), PE p-state warm-up matmuls under the first weight DMA, a
single 8-bank psum pool of [128,2,512] pair tiles (4-deep, so the PE never
waits on relu latency), relu split across ACT and DVE (GPSIMD cannot read
PSUM), and bf16 output (half the DMA bytes -> ~55us on the 360GB/s bus;
upcast on host) flushed every other row-pair, staggered by batch-block
parity, so the last transfer is a short 2-row tile.

Sharding: pure data parallel, batch 4096 -> 8 cores x 512.
"""

import sys

import numpy as np

sys.path.insert(0, "/opt/trn_rl_repo")

import ml_dtypes  # noqa: E402

import concourse.bass as bass  # noqa: E402
import concourse.bacc as bacc  # noqa: E402
import concourse.mybir as mybir  # noqa: E402
from concourse import tile  # noqa: E402
from concourse.bass_utils import run_bass_kernel_spmd  # noqa: E402

N_CORES = 8
B = 4096
BL = B // N_CORES  # 512 per core
F32 = mybir.dt.float32
F32R = mybir.dt.float32r
BF16 = mybir.dt.bfloat16
NPBF = ml_dtypes.bfloat16

# L4 feature windows per column group: group g of an output row (cols
# 128g..128(g+1), x-major) only reads input feats within these windows.
L4_WIN = ((0, 128), (80, 176), (128, 256))


# ---------------------------------------------------------------- host math
def _fused_matrices(Wup, Wc, sy, sx, Win, in_idx, out_idx, n_out_cols):
    """Compose convT(k=s=(sy,sx)) with 3x3 SAME conv into per-phase row
    matrices.  Returns {(p, delta): M} where out row y (p = y%sy, i = y//sy)
    accumulates  in_row[i+delta] @ M[(p, delta)]  over available deltas.
    x-edge clipping is baked into M; y-edge clipping == skipping absent rows.
    """
    Wup = np.asarray(Wup, np.float32)
    Wc = np.asarray(Wc, np.float32)
    Cin = Wup.shape[2]
    Wout = Win * sx
    mats = {}
    for p in range(sy):
        deltas = {0}
        if p == 0:
            deltas.add(-1)
        if p == sy - 1:
            deltas.add(1)
        for d in sorted(deltas):
            M = np.zeros((Win * Cin, n_out_cols), np.float32)
            y = sy + p  # representative interior row
            i_t = y // sy + d
            nz = False
            for dy in (-1, 0, 1):
                yp = y + dy
                if yp // sy != i_t:
                    continue
                py = yp % sy
                for x in range(Wout):
                    for dx in (-1, 0, 1):
                        xp = x + dx
                        if xp < 0 or xp >= Wout:
                            continue
                        j, qx = divmod(xp, sx)
                        # conv_transpose (transpose_kernel=False) applies the
                        # spatially mirrored kernel per phase
                        CC = Wup[sy - 1 - py, sx - 1 - qx] @ Wc[dy + 1, dx + 1]
                        M[np.ix_(in_idx(j), out_idx(x))] += CC
                        nz = True
            if nz:
                mats[(p, d)] = M
    return mats


def build_host_matrices(W_lin, W_up1, W_c1, W_up2, W_c2, W_up3, W_c3):
    # L2 input = h natural ordering: feat = c*8 + j   (c<32, j<8)
    r2 = _fused_matrices(
        W_up1, W_c1, 5, 2, 8,
        in_idx=lambda j: np.arange(32) * 8 + j,
        out_idx=lambda x: x * 16 + np.arange(16),
        n_out_cols=256,
    )
    # L3 input ordering: feat = j*16 + c ; output feat = x*8 + o
    r3 = _fused_matrices(
        W_up2, W_c2, 5, 2, 16,
        in_idx=lambda j: j * 16 + np.arange(16),
        out_idx=lambda x: x * 8 + np.arange(8),
        n_out_cols=256,
    )
    # L4 input ordering: feat = j*8 + c ; output col = x*6 + o  (x-major:
    # this makes each 128-col group read only a 128-feat j-window)
    r4 = _fused_matrices(
        W_up3, W_c3, 2, 2, 32,
        in_idx=lambda j: j * 8 + np.arange(8),
        out_idx=lambda x: x * 6 + np.arange(6),
        n_out_cols=384,
    )
    return np.asarray(W_lin, np.float32), r2, r3, r4


def _contribs(p, i, n_in_rows, mats, sy):
    out = []
    for d in (-1, 0, 1):
        if (p, d) in mats and 0 <= i + d < n_in_rows:
            out.append((i + d, mats[(p, d)]))
    return out


def numpy_forward(latent, W_lin, b_lin, r2, r3, r4):
    """Pure-numpy forward through the fused matrices (golden check)."""
    def leaky(x):
        return np.where(x > 0, x, 0.01 * x)

    h = leaky(latent.astype(np.float32) @ W_lin + b_lin)  # [B, 256]
    rows = h[:, None, :]  # [B, 1, 256]
    for (mats, sy, n_in) in ((r2, 5, 1), (r3, 5, 5)):
        nrows = n_in * sy
        out = np.zeros((h.shape[0], nrows, 256), np.float32)
        for y in range(nrows):
            i, p = divmod(y, sy)
            for (src, M) in _contribs(p, i, n_in, mats, sy):
                out[:, y] += rows[:, src] @ M
        rows = leaky(out)
    out = np.zeros((h.shape[0], 50, 384), np.float32)
    for y in range(50):
        i, p = divmod(y, 2)
        for (src, M) in _contribs(p, i, 25, r4, 2):
            out[:, y] += rows[:, src] @ M
    out = np.maximum(out, 0.0)
    # cols are x-major (x*6+o): [B, 50, 64, 6] -> NCHW [B, 6, 50, 64]
    return out.reshape(-1, 50, 64, 6).transpose(0, 3, 1, 2)


# keys in fixed order for weight-tile indexing
R3_KEYS = [(0, -1), (0, 0), (1, 0), (2, 0), (3, 0), (4, 0), (4, 1)]
R4_KEYS = [(0, -1), (0, 0), (1, 0), (1, 1)]


def _key_contribs(p, i, n_in, keys):
    out = []
    for d in (-1, 0, 1):
        if (p, d) in keys and 0 <= i + d < n_in:
            out.append((i + d, keys.index((p, d))))
    return out


# ---------------------------------------------------------------- bass build
_CACHED = {}


def build_nc():
    nc = bacc.Bacc('TRN2', target_bir_lowering=False, debug=False,
                   num_devices=N_CORES)

    # w1 (cols 0:256) and latent (cols 256:256+BL) share one DMA; row 4 is
    # (b_lin | ones) so the bias rides the contraction for free.
    wlat = nc.declare_dram_parameter("wlat", [5, 256 + BL], F32R, isOutput=False)
    # w2: (y, kc, mc) 128x128 blocks of the 5 R2 row matrices
    w2 = nc.declare_dram_parameter("w2", [128, 5, 2, 2, 128], BF16, isOutput=False)
    # w3: (mat, kc, mc) 128x128 blocks of the 7 R3 matrices
    w3 = nc.declare_dram_parameter("w3", [128, 7, 2, 2, 128], BF16, isOutput=False)
    # w4: (mat, group) feat-window x 128-col blocks of the 4 R4 matrices
    w4 = nc.declare_dram_parameter("w4", [128, 4, 3, 128], BF16, isOutput=False)
    # out stored (b, y, x*6+o) in bf16; host casts + transposes to NCHW
    out = nc.declare_dram_parameter("out", [BL, 50, 384], BF16, isOutput=True)

    LR = mybir.ActivationFunctionType.Lrelu
    RELU = mybir.ActivationFunctionType.Relu

    with tile.TileContext(nc) as tc:
        with (
            tc.tile_pool(name="wpool", bufs=1) as wp,
            tc.tile_pool(name="acts", bufs=1) as ap,
            tc.tile_pool(name="ps", bufs=4, space=bass.MemorySpace.PSUM) as pp,
            tc.tile_pool(name="outp", bufs=10) as op,
        ):
            wlat_t = wp.tile([5, 256 + BL], F32R, tag="wlat")
            nc.sync.dma_start(out=wlat_t[:], in_=wlat[:])
            w2_t = wp.tile([128, 5, 2, 2, 128], BF16, tag="w2")
            nc.sync.dma_start(out=w2_t[:], in_=w2[:])
            w3_t = wp.tile([128, 7, 2, 2, 128], BF16, tag="w3")
            nc.sync.dma_start(out=w3_t[:], in_=w3[:])
            w4_t = wp.tile([128, 4, 3, 128], BF16, tag="w4")
            nc.sync.dma_start(out=w4_t[:], in_=w4[:])

            # PE p-state warmup: matmuls on a zeroed tile start the frequency
            # ramp while the first weight DMA is still in flight.
            warm = wp.tile([128, 128], BF16, tag="warm")
            nc.gpsimd.memset(warm[:], 0.0)
            for _ in range(16):
                wps = pp.tile([128, 2, BL], F32, tag="ps")
                nc.tensor.matmul(wps[:, 0, 0:128], warm[:], warm[:],
                                 start=True, stop=True)

            # ---- L1: h[256, B] = leaky(W_lin.T @ lat + b)
            x1 = ap.tile([128, 2, BL], BF16, tag="x1")
            ps = pp.tile([128, 2, BL], F32, tag="ps")
            for mc in range(2):
                nc.tensor.matmul(
                    ps[:, mc, :], wlat_t[:, bass.ts(mc, 128)], wlat_t[:, 256:256 + BL],
                    start=True, stop=True,
                )
            for mc in range(2):
                nc.scalar.activation(x1[:, mc, :], ps[:, mc, :], LR, alpha=0.01)

            # ---- L2: 256 -> 1280 (5 rows x 256), input has 1 row
            x2 = ap.tile([128, 5, 2, BL], BF16, tag="x2")
            for y in range(5):
                ps = pp.tile([128, 2, BL], F32, tag="ps")
                for mc in range(2):
                    for kc in range(2):
                        nc.tensor.matmul(
                            ps[:, mc, :], w2_t[:, y, kc, mc, :], x1[:, kc, :],
                            start=(kc == 0), stop=(kc == 1),
                        )
                nc.scalar.activation(x2[:, y, :, :], ps[:, :, :], LR, alpha=0.01)

            # ---- L3: 1280 -> 6400 (25 rows x 256), 2-chunk contraction.
            # The pair-activation keeps ACT cheap; L4's middle feature
            # window (feats 80..176) is stitched by SBUF-to-SBUF DMA.
            x3 = ap.tile([128, 25, 2, BL], BF16, tag="x3")
            xm = ap.tile([96, 25, BL], BF16, tag="xm")  # feats 80..176
            for y in range(25):
                i, p = divmod(y, 5)
                cs = _key_contribs(p, i, 5, R3_KEYS)
                ps = pp.tile([128, 2, BL], F32, tag="ps")
                for mc in range(2):
                    n, tot = 0, len(cs) * 2
                    for (src, mi) in cs:
                        for kc in range(2):
                            nc.tensor.matmul(
                                ps[:, mc, :], w3_t[:, mi, kc, mc, :],
                                x2[:, src, kc, :],
                                start=(n == 0), stop=(n == tot - 1),
                            )
                            n += 1
                nc.scalar.activation(x3[:, y, :, :], ps[:, :, :], LR,
                                     alpha=0.01)
                if y % 5 == 4:
                    y0 = y - 4
                    nc.sync.dma_start(
                        out=xm[0:48, y0:y0 + 5, :],
                        in_=x3[80:128, y0:y0 + 5, 0, :])
                    nc.sync.dma_start(
                        out=xm[48:96, y0:y0 + 5, :],
                        in_=x3[0:48, y0:y0 + 5, 1, :])

            # ---- L4: 6400 -> 19200, batch-major psum pairs; relu split
            # ACT/DVE (GPSIMD cannot read PSUM); output flushed every other
            # pair, staggered by batch-block parity, so DMAs stay spread and
            # the kernel tail is a short transfer.
            relu_cycle = ("V", "A") * 6 + ("A",)
            n_relu = 0
            for bb in range(4):
                flushes = {p for p in range(25) if p % 2 == bb % 2} | {23, 24}
                ob, p0 = None, 0
                for pr in range(25):
                    if ob is None:
                        ob = op.tile([128, 4, 384], BF16, tag="ob",
                                     name=f"ob_{bb}_{pr}")
                        p0 = pr
                    ps = pp.tile([128, 2, 512], F32, tag="ps")
                    for yy in range(2):
                        y = 2 * pr + yy
                        i, p = divmod(y, 2)
                        cs = _key_contribs(p, i, 25, R4_KEYS)
                        for g in range(3):
                            n, tot = 0, len(cs)
                            for (src, mi) in cs:
                                if g == 0:
                                    lhs = x3[:, src, 0, bass.ts(bb, 128)]
                                    rhs = w4_t[:, mi, g, :]
                                elif g == 1:
                                    lhs = xm[:, src, bass.ts(bb, 128)]
                                    rhs = w4_t[0:96, mi, g, :]
                                else:
                                    lhs = x3[:, src, 1, bass.ts(bb, 128)]
                                    rhs = w4_t[:, mi, g, :]
                                nc.tensor.matmul(
                                    ps[:, yy, bass.ts(g, 128)], lhs, rhs,
                                    start=(n == 0), stop=(n == tot - 1),
                                )
                                n += 1
                    if bb == 3 and pr == 24:
                        eng = "V"  # keep the final relu off the busier ACT
                    else:
                        eng = relu_cycle[n_relu % len(relu_cycle)]
                        n_relu += 1
                    r0 = 2 * (pr - p0)
                    dst = ob[:, r0:r0 + 2, :]
                    src_ap = ps[:, :, 0:384]
                    if eng == "A":
                        nc.scalar.activation(dst, src_ap, RELU)
                    else:
                        nc.vector.tensor_scalar_max(dst, src_ap, 0.0)
                    if pr in flushes:
                        nrow = 2 * (pr - p0) + 2
                        nc.sync.dma_start(
                            out=out[bass.ts(bb, 128), 2 * p0:2 * p0 + nrow, :],
                            in_=ob[:, 0:nrow, :],
                        )
                        ob = None
    nc.compile()
    return nc


# ---------------------------------------------------------------- entry
def kernel(**inputs):
    latent = np.asarray(inputs["latent_vector"], np.float32)
    W_lin, r2, r3, r4 = build_host_matrices(
        inputs["W_lin"], inputs["W_up1"], inputs["W_c1"],
        inputs["W_up2"], inputs["W_c2"], inputs["W_up3"], inputs["W_c3"],
    )
    b_lin = np.asarray(inputs["b_lin"], np.float32)

    if "nc" not in _CACHED:
        _CACHED["nc"] = build_nc()
    nc = _CACHED["nc"]

    # w2/w3: [K=128, n, kc, mc, 128] layouts
    def pack_blocks(mats_list):
        n = len(mats_list)
        t = np.zeros((128, n, 2, 2, 128), np.float32)
        for mi, M in enumerate(mats_list):
            for kc in range(2):
                for mc in range(2):
                    t[:, mi, kc, mc, :] = M[kc * 128:(kc + 1) * 128,
                                            mc * 128:(mc + 1) * 128]
        return np.ascontiguousarray(t.astype(NPBF))

    w2_host = pack_blocks([r2[(p, 0)] for p in range(5)])

    w3_host = pack_blocks([r3[k] for k in R3_KEYS])

    # w4: [128, 4, 3, 128]; group g takes rows L4_WIN[g], cols 128g..128(g+1)
    w4_host = np.zeros((128, 4, 3, 128), np.float32)
    for mi, k in enumerate(R4_KEYS):
        M = r4[k]
        for g, (r0, r1) in enumerate(L4_WIN):
            blk = M[r0:r1, g * 128:(g + 1) * 128]
            # sanity: all nonzeros of this col-group live inside the window
            rest = M[:, g * 128:(g + 1) * 128].copy()
            rest[r0:r1] = 0.0
            assert np.all(rest == 0.0), f"L4 window violated mat {k} group {g}"
            w4_host[:r1 - r0, mi, g, :] = blk
    w4_host = np.ascontiguousarray(w4_host.astype(NPBF))

    base = {"w2": w2_host, "w3": w3_host, "w4": w4_host}

    w1b = np.concatenate([W_lin, b_lin[None, :]], axis=0)  # [5, 256]
    in_maps = []
    for c in range(N_CORES):
        sh = latent[c * BL:(c + 1) * BL]
        lat1 = np.concatenate(
            [sh.T, np.ones((1, BL), np.float32)], axis=0)  # [5, BL]
        wlat = np.concatenate([w1b, lat1], axis=1)
        in_maps.append({**base,
                        "wlat": np.ascontiguousarray(wlat)})

    _CACHED["maps"] = in_maps
    res = run_bass_kernel_spmd(nc, in_maps, list(range(N_CORES)))
    outs = [
        np.asarray(r["out"], NPBF).astype(np.float32)
        .reshape(BL, 50, 64, 6).transpose(0, 3, 1, 2)
        for r in res.results
    ]
    return np.ascontiguousarray(np.concatenate(outs, axis=0))


if __name__ == "__main__":
    rng = np.random.default_rng(0)
    fake = {
        "latent_vector": rng.standard_normal((B, 4)).astype(np.float32),
        "W_lin": rng.standard_normal((4, 256)).astype(np.float32) * 0.5,
        "b_lin": np.zeros(256, np.float32),
        "W_up1": rng.standard_normal((5, 2, 32, 32)).astype(np.float32) * 0.1,
        "W_c1": rng.standard_normal((3, 3, 32, 16)).astype(np.float32) * 0.1,
        "W_up2": rng.standard_normal((5, 2, 16, 16)).astype(np.float32) * 0.1,
        "W_c2": rng.standard_normal((3, 3, 16, 8)).astype(np.float32) * 0.1,
        "W_up3": rng.standard_normal((2, 2, 8, 8)).astype(np.float32) * 0.1,
        "W_c3": rng.standard_normal((3, 3, 8, 6)).astype(np.float32) * 0.1,
    }
    o = kernel(**fake)
    print("kernel out", o.shape, o.dtype)

